# revision 2
# baseline (speedup 1.0000x reference)
"""Causal single-head attention on 8 Trainium2 NeuronCores.

Problem: x[4, 2048, 1024] fp32, Wq/Wk/Wv[1024, 1024] fp32.
  q,k,v = x@Wq, x@Wk, x@Wv ; out = softmax(mask(q k^T)/32) @ v

Sharding (SPMD — one program, 8 cores, per-core data):
  core = 2*b + h  handles batch b, queries {t : t % 2 == h} (1024 queries).
  The interleaved (mod-2) query split makes the causal block structure
  shape-identical across cores.

K/V projections are pair-split by interleaved 256-token blocks: global
block i (tokens [256i, 256i+256)) is projected by pair-rank i % 2.  Each
512-token window g's K^T and V halves are packed into ONE DRAM tile and
pair-AllGathered (4 gathers of 1 MB payload each).  The gathers are
emitted right after window g's projection so arrival order (g=0..3)
matches the causal consumption order (q-block jb needs windows <= jb):
gather g lands ~g*20us apart while attention blocks consume them
~(6..13)us apart with a ~45us head start.  All K_sb/V_sb content is
unpacked from the gather output (uniform SPMD addressing: slot r of
gather g holds global block 2g+r regardless of own rank).

Layouts:
  - host passes x^T slices ([D, tokens]) so K^T/Q^T come out of matmuls
    with no transposes; V is projected as [token, d] tiles.
  - scores are computed transposed ([k, q]) so expS feeds the attn @ V
    matmul directly as the stationary operand.
  - causal trim: for the first q-sub (s=0, 128 queries) of each q-block
    the last two diagonal k-tiles are entirely invisible, so the context
    matmul and the denominator adds skip them (the scores themselves are
    not trimmed: N=128 matmuls are LDWEIGHTS-bound, trimming buys 0).
  - softmax denominator: DVE accumulates partition-partial sums, one tiny
    fp32 ones-matmul per q-sub reduces across partitions.  No
    max-subtraction (logits/32 are ~N(0, 0.41^2); exp never overflows).

Dtypes: bf16 matmul inputs everywhere (fp32 PSUM accum), bf16 expS / V
context matmuls, fp32 softmax denominator / normalization.  Optional
K8=True stores K^T (and Q^T) in fp8-e4m3 to shrink the gather payload.

Measured on HW: see test.py; baseline (V projected fully on every core,
fp32r context matmuls) was ~210-245us depending on board throttling.
"""

import os
import numpy as np
import ml_dtypes

import concourse.mybir as mybir
import concourse.tile as tile
from concourse import bacc

F32 = mybir.dt.float32
BF16 = mybir.dt.bfloat16
FP8 = mybir.dt.float8e4
BF16_NP = ml_dtypes.bfloat16

B, T, D = 4, 2048, 1024
P = 128
DC = D // P          # 8 contraction chunks
NT = T // P          # 16 key tiles
NG = 4               # gathers == 512-token windows
QB = 256             # queries per q-block (per core)
NJB = (T // 2) // QB # 4 q-blocks per core
SCALE = 1.0 / 32.0   # 1/sqrt(D)

MASK_NEG = -1.0e9
# K8: ship the K^T gather payload (and the score matmuls, K and Q both)
# in fp8-e4m3.  Shrinks each gather from 1 MB to 0.75 MB when collective
# bandwidth is the bottleneck; costs ~1% relative error (vs ~0.4% bf16).
K8 = False
K_DT = FP8 if K8 else BF16
PAIRS = [[0, 1], [2, 3], [4, 5], [6, 7]]
_EXP = mybir.ActivationFunctionType.Exp

# bytes per element of K payload (for rearrange factor bookkeeping)
_KT_COLS = 256 if not K8 else 128  # K slot viewed as bf16-sized elements


def _emit(nc, tc, xTk_d, xTq_d, wq_d, wk_d, wv_d, masks_d, out_d):
    HT = T // 2  # queries per core

    def mm(out, lhsT, rhs, start, stop, **kw):
        nc.tensor.matmul(out, lhsT, rhs, start=start, stop=stop, **kw)

    with (
        tc.sbuf_pool(name="persist", bufs=1) as persist,
        tc.psum_pool(name="p512", bufs=3) as p512,
        tc.psum_pool(name="p256", bufs=4) as p256,
        tc.psum_pool(name="pden", bufs=1) as pden,
    ):
        # ---- persistent SBUF tensors ----
        K_sb = persist.tile([P, DC * T], K_DT, tag="K", name="K_sb")
        V_sb = persist.tile([P, NT * D], BF16, tag="V", name="V_sb")
        Q_sb = persist.tile([P, DC * HT], K_DT, tag="Q", name="Q_sb")

        # ---- projections: K^T and V pair-split by 256-token blocks ----
        # Window g: global blocks {2g, 2g+1}; this core projects block
        # 2g + (pair rank) — its x slice arrives pre-packed in xTk col
        # range [256g, 256g+256).  One packed KV AllGather per window.
        with (
            tc.sbuf_pool(name="wkvp", bufs=1) as wkv_pool,
            tc.sbuf_pool(name="xtwp", bufs=2) as xtw_pool,
            tc.sbuf_pool(name="stgp", bufs=4) as stg_pool,
            tc.tile_pool(name="drp", bufs=1, space="DRAM") as dr_pool,
            nc.named_scope("kv_proj"),
        ):
            wk_sb = wkv_pool.tile([P, DC * D], BF16, tag="wk", name="wk_sb")
            wv_sb = wkv_pool.tile([P, DC * D], BF16, tag="wv", name="wv_sb")
            for c in range(DC):
                nc.sync.dma_start(out=wk_sb[:, c * D:(c + 1) * D],
                                  in_=wk_d[c * P:(c + 1) * P, :])
            xtks = [None, None]
            for g in range(NG):
                w, half = divmod(g, 2)
                if half == 0:
                    xtk = xtw_pool.tile([P, DC * 512], BF16, tag="xtw",
                                        name="xtk")
                    for c in range(DC):
                        nc.sync.dma_start(
                            out=xtk[:, c * 512:(c + 1) * 512],
                            in_=xTk_d[c * P:(c + 1) * P, 512 * w:512 * (w + 1)])
                    xtks[w] = xtk
                xtk = xtks[w]
                xo = 256 * half  # col offset of this block in the window tile
                # K^T of own block -> staging (K_DT)
                stK = stg_pool.tile([P, DC * 256], K_DT, tag="stK", name="stK")
                for c2 in range(DC):
                    ps = p256.tile([P, QB], F32, tag="mm256", name="ps_k")
                    for c in range(DC):
                        mm(ps, wk_sb[:, c * D + P * c2: c * D + P * (c2 + 1)],
                           xtk[:, c * 512 + xo: c * 512 + xo + 256],
                           c == 0, c == DC - 1)
                    nc.scalar.copy(out=stK[:, c2 * 256:(c2 + 1) * 256], in_=ps)
                if g == 0:
                    for c in range(DC):
                        nc.sync.dma_start(out=wv_sb[:, c * D:(c + 1) * D],
                                          in_=wv_d[c * P:(c + 1) * P, :])
                # V of own block -> staging (bf16), [token, d] layout
                stV = stg_pool.tile([P, 2 * D], BF16, tag="stV", name="stV")
                for ts in range(2):
                    for n in range(2):
                        ps = p512.tile([P, 512], F32, tag="mm512", name="ps_v")
                        for c in range(DC):
                            mm(ps,
                               xtk[:, c * 512 + xo + P * ts:
                                   c * 512 + xo + P * (ts + 1)],
                               wv_sb[:, c * D + 512 * n: c * D + 512 * (n + 1)],
                               c == 0, c == DC - 1)
                        nc.scalar.copy(
                            out=stV[:, ts * D + 512 * n: ts * D + 512 * (n + 1)],
                            in_=ps)
                # pack K^T + V into one DRAM tile, pair-AllGather.
                # kv[0] flat = K^T row-major [D, 256] (in K_DT elements,
                # bitcast to bf16-sized units for uniform tile dtype);
                # kv[1] flat = V row-major [256, D] bf16.
                kc = _KT_COLS
                kv = dr_pool.tile([2, 256 * 1024], BF16, tag=f"kv{g}",
                                  name=f"kv{g}")
                stK_b = stK[:].bitcast(BF16)  # [P, DC*kc]
                nc.sync.dma_start(
                    out=kv[0, :DC * P * kc]
                        .rearrange("(c p t) -> p c t", c=DC, p=P, t=kc),
                    in_=stK_b.rearrange("p (c t) -> p c t", c=DC))
                nc.sync.dma_start(
                    out=kv[1].rearrange("(t p d) -> p t d", t=2, p=P, d=D),
                    in_=stV[:].rearrange("p (t d) -> p t d", t=2))
                kgs = dr_pool.tile([2, 2, 256 * 1024], BF16, tag=f"kg{g}",
                                   name=f"kg{g}")
                nc.gpsimd.collective_compute(
                    "AllGather", mybir.AluOpType.bypass,
                    replica_groups=PAIRS, ins=[kv[:]], outs=[kgs[:]])
                # unpack both pair slots: slot r holds global block 2g+r
                K_vw = K_sb[:].bitcast(BF16).rearrange(
                    "p (c t) -> p c t", c=DC)  # t-dim: T in K_DT units
                ktc = T * _KT_COLS // 256  # bf16-unit cols per c2 row-block
                for r in range(2):
                    Bg = 2 * g + r
                    nc.sync.dma_start(
                        out=K_vw[:, :, Bg * kc: (Bg + 1) * kc],
                        in_=kgs[r, 0, :DC * P * kc]
                            .rearrange("(c p t) -> p c t", c=DC, p=P, t=kc))
                    nc.sync.dma_start(
                        out=V_sb[:, 2 * Bg * D: (2 * Bg + 2) * D]
                            .rearrange("p (t d) -> p t d", t=2),
                        in_=kgs[r, 1].rearrange("(t p d) -> p t d",
                                                t=2, p=P, d=D))

            # ---- Q^T projection (own queries) ----
            wq_sb = wkv_pool.tile([P, DC * D], BF16, tag="wq", name="wq_sb")
            for c in range(DC):
                nc.sync.dma_start(out=wq_sb[:, c * D:(c + 1) * D],
                                  in_=wq_d[c * P:(c + 1) * P, :])
            xtqs = []
            for jp in range(NJB // 2):
                xtq = xtw_pool.tile([P, DC * 512], BF16, tag="xtq",
                                    name="xtq")
                for c in range(DC):
                    nc.sync.dma_start(
                        out=xtq[:, c * 512:(c + 1) * 512],
                        in_=xTq_d[c * P:(c + 1) * P, 512 * jp:512 * (jp + 1)])
                xtqs.append(xtq)
            with nc.named_scope("q_proj"):
                for jp in range(NJB // 2):
                    xtq = xtqs[jp]
                    for c2 in range(DC):
                        ps = p512.tile([P, 512], F32, tag="mm512", name="ps_q")
                        for c in range(DC):
                            mm(ps,
                               wq_sb[:, c * D + P * c2: c * D + P * (c2 + 1)],
                               xtq[:, c * 512:(c + 1) * 512], c == 0,
                               c == DC - 1)
                        nc.scalar.copy(
                            out=Q_sb[:, c2 * HT + 512 * jp:
                                     c2 * HT + 512 * (jp + 1)],
                            in_=ps)

        # ---- attention, per q-block ----
        with (
            tc.sbuf_pool(name="attnp", bufs=1) as attnp,
            tc.sbuf_pool(name="recipp", bufs=2) as recip_pool,
            tc.sbuf_pool(name="accp", bufs=2) as acc_pool,
            tc.sbuf_pool(name="outp", bufs=4) as out_pool,
            nc.named_scope("attn"),
        ):
            expS = attnp.tile([P, NT * QB], BF16, tag="E", name="expS")
            mask_sb = attnp.tile([P, 4 * QB], F32, tag="M", name="mask_sb")
            ones_f32 = attnp.tile([P, 1], F32, tag="O32", name="ones_f32")
            nc.vector.memset(ones_f32, 1.0)
            for u in range(4):
                nc.sync.dma_start(out=mask_sb[:, u * QB:(u + 1) * QB],
                                  in_=masks_d[u])
            for jb in range(NJB):
                kt = 4 * (jb + 1)  # k-tiles needed by this q-block
                # pass 1: scores^T -> exp (-> mask on the 4 diagonal tiles)
                for t in range(kt):
                    ps = p256.tile([P, QB], F32, tag="mm256", name="ps_s")
                    for c in range(DC):
                        mm(ps,
                           K_sb[:, c * T + P * t: c * T + P * (t + 1)],
                           Q_sb[:, c * HT + QB * jb: c * HT + QB * (jb + 1)],
                           c == 0, c == DC - 1)
                    if t >= kt - 4:
                        u = t - (kt - 4)
                        nc.vector.tensor_add(ps, ps,
                                             mask_sb[:, u * QB:(u + 1) * QB])
                    nc.scalar.activation(out=expS[:, t * QB:(t + 1) * QB],
                                         in_=ps, func=_EXP, scale=SCALE)
                # denominators: den[q, s] = sum_k expS[k, q].  The last two
                # diagonal k-tiles are fully masked for s=0, so their s=0
                # halves are skipped (and never read by the context pass).
                acc = acc_pool.tile([P, QB], F32, tag="acc", name="acc")
                nc.vector.tensor_copy(acc, expS[:, 0:QB])
                for t in range(1, kt):
                    if t < kt - 2:
                        nc.vector.tensor_add(
                            acc, acc, expS[:, t * QB:(t + 1) * QB])
                    else:
                        nc.vector.tensor_add(
                            acc[:, P:QB], acc[:, P:QB],
                            expS[:, t * QB + P:(t + 1) * QB])
                den = pden.tile([P, 2], F32, tag="den", name="den")
                for s in range(2):
                    nc.tensor.matmul(den[:, s:s + 1],
                                     acc[:, P * s:P * (s + 1)], ones_f32,
                                     start=True, stop=True,
                                     skip_group_check=True)
                recip = recip_pool.tile([P, 2], F32, tag="recip", name="recip")
                nc.vector.reciprocal(recip, den)
                # pass 2: ctx[q, d] = sum_k expS[k, q] * V[k, d], normalize.
                # s=0 skips the last two (fully masked) diagonal k-tiles.
                for s in range(2):
                    nkt = kt - 2 if s == 0 else kt
                    for n in range(2):
                        ps = p512.tile([P, 512], F32, tag="mm512", name="ps_c")
                        for t in range(nkt):
                            mm(ps,
                               expS[:, t * QB + P * s: t * QB + P * (s + 1)],
                               V_sb[:, t * D + 512 * n: t * D + 512 * (n + 1)],
                               t == 0, t == nkt - 1)
                        ot = out_pool.tile([P, 512], F32, tag="out", name="ot")
                        nc.vector.tensor_scalar_mul(ot, ps, recip[:, s:s + 1])
                        nc.sync.dma_start(
                            out=out_d[QB * jb + P * s: QB * jb + P * (s + 1),
                                      512 * n: 512 * (n + 1)],
                            in_=ot)


def build_nc():
    nc = bacc.Bacc("TRN2", target_bir_lowering=False, debug=False,
                   num_devices=8)
    xTk_d = nc.dram_tensor("xTk", [D, T // 2], BF16, kind="ExternalInput")
    xTq_d = nc.dram_tensor("xTq", [D, T // 2], BF16, kind="ExternalInput")
    wq_d = nc.dram_tensor("wq", [D, D], BF16, kind="ExternalInput")
    wk_d = nc.dram_tensor("wk", [D, D], BF16, kind="ExternalInput")
    wv_d = nc.dram_tensor("wv", [D, D], BF16, kind="ExternalInput")
    masks_d = nc.dram_tensor("masks", [4, P, QB], F32, kind="ExternalInput")
    out_d = nc.dram_tensor("out", [T // 2, D], F32, kind="ExternalOutput")
    with tile.TileContext(nc) as tc:
        _emit(nc, tc, xTk_d[:], xTq_d[:], wq_d[:], wk_d[:], wv_d[:],
              masks_d[:], out_d[:])
    nc.compile()
    return nc


def make_masks(h):
    """Additive causal mask: 0 where key (128u + p) <= query (2j + h), else
    -1e9, within a 512-position diagonal window (positions relative to the
    q-block base).  Applied to raw scores before exp."""
    u = np.arange(4)[:, None, None]
    p = np.arange(P)[None, :, None]
    j = np.arange(QB)[None, None, :]
    vis = (128 * u + p <= 2 * j + h)
    return np.where(vis, 0.0, MASK_NEG).astype(np.float32)


def make_in_maps(x, W_query, W_key, W_value):
    wq = np.ascontiguousarray(W_query).astype(BF16_NP)
    wk = np.ascontiguousarray(W_key).astype(BF16_NP)
    wv = np.ascontiguousarray(W_value).astype(BF16_NP)
    masks = [make_masks(h) for h in range(2)]
    in_maps = []
    for core in range(8):
        b, h = divmod(core, 2)
        xb = np.asarray(x[b], dtype=np.float32)
        # own 256-token blocks (2g + h for g in 0..3), packed contiguously
        own = np.concatenate(
            [xb[256 * (2 * g + h): 256 * (2 * g + h) + 256] for g in range(4)],
            axis=0)
        in_maps.append({
            "xTk": np.ascontiguousarray(own.T).astype(BF16_NP),
            "xTq": np.ascontiguousarray(xb[h::2].T).astype(BF16_NP),
            "wq": wq, "wk": wk, "wv": wv,
            "masks": masks[h],
        })
    return in_maps


_NC_CACHE = {}
LAST_EXEC_NS = None


def kernel(x, W_query, W_key, W_value):
    global LAST_EXEC_NS
    from concourse.bass_utils import run_bass_kernel_spmd

    if "nc" not in _NC_CACHE:
        _NC_CACHE["nc"] = build_nc()
    nc = _NC_CACHE["nc"]

    in_maps = make_in_maps(x, W_query, W_key, W_value)
    trace = bool(os.environ.get("BASS_TRACE"))
    res = run_bass_kernel_spmd(nc, in_maps, core_ids=list(range(8)),
                               trace=trace)
    LAST_EXEC_NS = res.exec_time_ns

    out = np.empty((B, T, D), dtype=np.float32)
    for core in range(8):
        b, h = divmod(core, 2)
        out[b, h::2, :] = res.results[core]["out"]
    return out


if __name__ == "__main__":
    import time
    t0 = time.time()
    nc = build_nc()
    print(f"build+compile took {time.time() - t0:.1f}s")
    print("built ok")


# revision 4
# speedup vs baseline: 1.0233x; 1.0233x over previous
"""Causal single-head attention on 8 Trainium2 NeuronCores.

Problem: x[4, 2048, 1024] fp32, Wq/Wk/Wv[1024, 1024] fp32.
  q,k,v = x@Wq, x@Wk, x@Wv ; out = softmax(mask(q k^T)/32) @ v

Sharding (SPMD — one program, 8 cores, per-core data):
  core = 2*b + h  handles batch b, queries {t : t % 2 == h} (1024 queries).
  The interleaved (mod-2) query split makes the causal block structure
  shape-identical across cores.

K/V projections are pair-split by interleaved 256-token blocks: global
block i (tokens [256i, 256i+256)) is projected by pair-rank i % 2.  Each
512-token window g's K^T and V halves are packed into ONE DRAM tile and
pair-AllGathered (4 gathers of 1 MB payload each).  The gathers are
emitted right after window g's projection so arrival order (g=0..3)
matches the causal consumption order (q-block jb needs windows <= jb):
gather g lands ~g*20us apart while attention blocks consume them
~(6..13)us apart with a ~45us head start.  All K_sb/V_sb content is
unpacked from the gather output (uniform SPMD addressing: slot r of
gather g holds global block 2g+r regardless of own rank).

Layouts:
  - host passes x^T slices ([D, tokens]) so K^T/Q^T come out of matmuls
    with no transposes; V is projected as [token, d] tiles.
  - scores are computed transposed ([k, q]) so expS feeds the attn @ V
    matmul directly as the stationary operand.
  - causal trim: for the first q-sub (s=0, 128 queries) of each q-block
    the last two diagonal k-tiles are entirely invisible, so the context
    matmul and the denominator adds skip them (the scores themselves are
    not trimmed: N=128 matmuls are LDWEIGHTS-bound, trimming buys 0).
  - softmax denominator: DVE accumulates partition-partial sums, one tiny
    fp32 ones-matmul per q-sub reduces across partitions.  No
    max-subtraction (logits/32 are ~N(0, 0.41^2); exp never overflows).

Dtypes: bf16 matmul inputs everywhere (fp32 PSUM accum), bf16 expS / V
context matmuls, fp32 softmax denominator / normalization.  Optional
K8=True stores K^T (and Q^T) in fp8-e4m3 to shrink the gather payload.

Measured on HW: see test.py; baseline (V projected fully on every core,
fp32r context matmuls) was ~210-245us depending on board throttling.
"""

import os
import numpy as np
import ml_dtypes

import concourse.mybir as mybir
import concourse.tile as tile
from concourse import bacc

F32 = mybir.dt.float32
BF16 = mybir.dt.bfloat16
FP8 = mybir.dt.float8e4
BF16_NP = ml_dtypes.bfloat16

B, T, D = 4, 2048, 1024
P = 128
DC = D // P          # 8 contraction chunks
NT = T // P          # 16 key tiles
NG = 4               # gathers == 512-token windows
QB = 256             # queries per q-block (per core)
NJB = (T // 2) // QB # 4 q-blocks per core
SCALE = 1.0 / 32.0   # 1/sqrt(D)

MASK_NEG = -1.0e9
# K8: ship the K^T gather payload (and the score matmuls, K and Q both)
# in fp8-e4m3.  Shrinks each gather from 1 MB to 0.75 MB when collective
# bandwidth is the bottleneck; costs ~1% relative error (vs ~0.4% bf16).
K8 = False
K_DT = FP8 if K8 else BF16
PAIRS = [[0, 1], [2, 3], [4, 5], [6, 7]]
_EXP = mybir.ActivationFunctionType.Exp

# bytes per element of K payload (for rearrange factor bookkeeping)
_KT_COLS = 256 if not K8 else 128  # K slot viewed as bf16-sized elements


def _emit(nc, tc, xTk_d, xTq_d, wq_d, wk_d, wv_d, masks_d, out_d):
    HT = T // 2  # queries per core

    def mm(out, lhsT, rhs, start, stop, **kw):
        nc.tensor.matmul(out, lhsT, rhs, start=start, stop=stop, **kw)

    with (
        tc.sbuf_pool(name="persist", bufs=1) as persist,
        tc.psum_pool(name="p512", bufs=3) as p512,
        tc.psum_pool(name="p256", bufs=4) as p256,
        tc.psum_pool(name="pden", bufs=1) as pden,
    ):
        # ---- persistent SBUF tensors ----
        K_sb = persist.tile([P, DC * T], K_DT, tag="K", name="K_sb")
        V_sb = persist.tile([P, NT * D], BF16, tag="V", name="V_sb")
        Q_sb = persist.tile([P, DC * HT], K_DT, tag="Q", name="Q_sb")

        # ---- projections: K^T and V pair-split by 256-token blocks ----
        # Window g: global blocks {2g, 2g+1}; this core projects block
        # 2g + (pair rank) — its x slice arrives pre-packed in xTk col
        # range [256g, 256g+256).  One packed KV AllGather per window.
        with (
            tc.sbuf_pool(name="wkvp", bufs=1) as wkv_pool,
            tc.sbuf_pool(name="xtwp", bufs=2) as xtw_pool,
            tc.sbuf_pool(name="stgp", bufs=4) as stg_pool,
            tc.tile_pool(name="drp", bufs=1, space="DRAM") as dr_pool,
            nc.named_scope("kv_proj"),
        ):
            wk_sb = wkv_pool.tile([P, DC * D], BF16, tag="wk", name="wk_sb")
            wv_sb = wkv_pool.tile([P, DC * D], BF16, tag="wv", name="wv_sb")
            for c in range(DC):
                nc.sync.dma_start(out=wk_sb[:, c * D:(c + 1) * D],
                                  in_=wk_d[c * P:(c + 1) * P, :])
            xtks = [None, None]
            kgss = []
            for g in range(NG):
                w, half = divmod(g, 2)
                if half == 0:
                    xtk = xtw_pool.tile([P, DC * 512], BF16, tag="xtw",
                                        name="xtk")
                    for c in range(DC):
                        nc.sync.dma_start(
                            out=xtk[:, c * 512:(c + 1) * 512],
                            in_=xTk_d[c * P:(c + 1) * P, 512 * w:512 * (w + 1)])
                    xtks[w] = xtk
                xtk = xtks[w]
                xo = 256 * half  # col offset of this block in the window tile
                # K^T of own block -> staging (K_DT)
                stK = stg_pool.tile([P, DC * 256], K_DT, tag="stK", name="stK")
                for c2 in range(DC):
                    ps = p256.tile([P, QB], F32, tag="mm256", name="ps_k")
                    for c in range(DC):
                        mm(ps, wk_sb[:, c * D + P * c2: c * D + P * (c2 + 1)],
                           xtk[:, c * 512 + xo: c * 512 + xo + 256],
                           c == 0, c == DC - 1)
                    nc.scalar.copy(out=stK[:, c2 * 256:(c2 + 1) * 256], in_=ps)
                if g == 0:
                    for c in range(DC):
                        nc.sync.dma_start(out=wv_sb[:, c * D:(c + 1) * D],
                                          in_=wv_d[c * P:(c + 1) * P, :])
                # V of own block -> staging (bf16), [token, d] layout
                stV = stg_pool.tile([P, 2 * D], BF16, tag="stV", name="stV")
                for ts in range(2):
                    for n in range(2):
                        ps = p512.tile([P, 512], F32, tag="mm512", name="ps_v")
                        for c in range(DC):
                            mm(ps,
                               xtk[:, c * 512 + xo + P * ts:
                                   c * 512 + xo + P * (ts + 1)],
                               wv_sb[:, c * D + 512 * n: c * D + 512 * (n + 1)],
                               c == 0, c == DC - 1)
                        nc.scalar.copy(
                            out=stV[:, ts * D + 512 * n: ts * D + 512 * (n + 1)],
                            in_=ps)
                # pack K^T + V into one DRAM tile, pair-AllGather.
                # kv[0] flat = K^T row-major [D, 256] (in K_DT elements,
                # bitcast to bf16-sized units for uniform tile dtype);
                # kv[1] flat = V row-major [256, D] bf16.
                # Staging rides the Activation HWDGE ring (nc.scalar) —
                # the Sync ring is busy streaming input loads at this
                # point and would delay the latency-critical gathers.
                kc = _KT_COLS
                kv = dr_pool.tile([2, 256 * 1024], BF16, tag=f"kv{g}",
                                  name=f"kv{g}")
                stK_b = stK[:].bitcast(BF16)  # [P, DC*kc]
                nc.scalar.dma_start(
                    out=kv[0, :DC * P * kc]
                        .rearrange("(c p t) -> p c t", c=DC, p=P, t=kc),
                    in_=stK_b.rearrange("p (c t) -> p c t", c=DC))
                nc.scalar.dma_start(
                    out=kv[1].rearrange("(t p d) -> p t d", t=2, p=P, d=D),
                    in_=stV[:].rearrange("p (t d) -> p t d", t=2))
                kgs = dr_pool.tile([2, 2, 256 * 1024], BF16, tag=f"kg{g}",
                                   name=f"kg{g}")
                kgss.append(kgs)
                nc.gpsimd.collective_compute(
                    "AllGather", mybir.AluOpType.bypass,
                    replica_groups=PAIRS, ins=[kv[:]], outs=[kgs[:]])

            # unpack both pair slots of every gather: slot r of gather g
            # holds global block 2g+r (uniform SPMD addressing).  These
            # ride GpSimd's queue: an unpack trigger sem-waits on its
            # gather, which must not block the Sync ring (input loads /
            # output stores) or Scalar (PSUM->SBUF copies, exp).  K slots
            # first — scores need K before context needs V.
            K_vw = K_sb[:].bitcast(BF16).rearrange(
                "p (c t) -> p c t", c=DC)  # t-dim: T in K_DT units
            kc = _KT_COLS
            for g in range(NG):
                for r in range(2):
                    Bg = 2 * g + r
                    nc.gpsimd.dma_start(
                        out=K_vw[:, :, Bg * kc: (Bg + 1) * kc],
                        in_=kgss[g][r, 0, :DC * P * kc]
                            .rearrange("(c p t) -> p c t", c=DC, p=P, t=kc))
                for r in range(2):
                    Bg = 2 * g + r
                    nc.gpsimd.dma_start(
                        out=V_sb[:, 2 * Bg * D: (2 * Bg + 2) * D]
                            .rearrange("p (t d) -> p t d", t=2),
                        in_=kgss[g][r, 1].rearrange("(t p d) -> p t d",
                                                    t=2, p=P, d=D))

            # ---- Q^T projection (own queries) ----
            wq_sb = wkv_pool.tile([P, DC * D], BF16, tag="wq", name="wq_sb")
            for c in range(DC):
                nc.sync.dma_start(out=wq_sb[:, c * D:(c + 1) * D],
                                  in_=wq_d[c * P:(c + 1) * P, :])
            xtqs = []
            for jp in range(NJB // 2):
                xtq = xtw_pool.tile([P, DC * 512], BF16, tag="xtq",
                                    name="xtq")
                for c in range(DC):
                    nc.sync.dma_start(
                        out=xtq[:, c * 512:(c + 1) * 512],
                        in_=xTq_d[c * P:(c + 1) * P, 512 * jp:512 * (jp + 1)])
                xtqs.append(xtq)
            with nc.named_scope("q_proj"):
                for jp in range(NJB // 2):
                    xtq = xtqs[jp]
                    for c2 in range(DC):
                        ps = p512.tile([P, 512], F32, tag="mm512", name="ps_q")
                        for c in range(DC):
                            mm(ps,
                               wq_sb[:, c * D + P * c2: c * D + P * (c2 + 1)],
                               xtq[:, c * 512:(c + 1) * 512], c == 0,
                               c == DC - 1)
                        nc.scalar.copy(
                            out=Q_sb[:, c2 * HT + 512 * jp:
                                     c2 * HT + 512 * (jp + 1)],
                            in_=ps)

        # ---- attention, per q-block ----
        with (
            tc.sbuf_pool(name="attnp", bufs=1) as attnp,
            tc.sbuf_pool(name="recipp", bufs=2) as recip_pool,
            tc.sbuf_pool(name="accp", bufs=2) as acc_pool,
            tc.sbuf_pool(name="outp", bufs=4) as out_pool,
            nc.named_scope("attn"),
        ):
            expS = attnp.tile([P, NT * QB], BF16, tag="E", name="expS")
            mask_sb = attnp.tile([P, 4 * QB], F32, tag="M", name="mask_sb")
            ones_f32 = attnp.tile([P, 1], F32, tag="O32", name="ones_f32")
            nc.vector.memset(ones_f32, 1.0)
            for u in range(4):
                nc.sync.dma_start(out=mask_sb[:, u * QB:(u + 1) * QB],
                                  in_=masks_d[u])
            for jb in range(NJB):
                kt = 4 * (jb + 1)  # k-tiles needed by this q-block
                # pass 1: scores^T -> exp (-> mask on the 4 diagonal tiles)
                for t in range(kt):
                    ps = p256.tile([P, QB], F32, tag="mm256", name="ps_s")
                    for c in range(DC):
                        mm(ps,
                           K_sb[:, c * T + P * t: c * T + P * (t + 1)],
                           Q_sb[:, c * HT + QB * jb: c * HT + QB * (jb + 1)],
                           c == 0, c == DC - 1)
                    if t >= kt - 4:
                        u = t - (kt - 4)
                        nc.vector.tensor_add(ps, ps,
                                             mask_sb[:, u * QB:(u + 1) * QB])
                    nc.scalar.activation(out=expS[:, t * QB:(t + 1) * QB],
                                         in_=ps, func=_EXP, scale=SCALE)
                # denominators: den[q, s] = sum_k expS[k, q].  The last two
                # diagonal k-tiles are fully masked for s=0, so their s=0
                # halves are skipped (and never read by the context pass).
                acc = acc_pool.tile([P, QB], F32, tag="acc", name="acc")
                nc.vector.tensor_copy(acc, expS[:, 0:QB])
                for t in range(1, kt):
                    if t < kt - 2:
                        nc.vector.tensor_add(
                            acc, acc, expS[:, t * QB:(t + 1) * QB])
                    else:
                        nc.vector.tensor_add(
                            acc[:, P:QB], acc[:, P:QB],
                            expS[:, t * QB + P:(t + 1) * QB])
                den = pden.tile([P, 2], F32, tag="den", name="den")
                for s in range(2):
                    nc.tensor.matmul(den[:, s:s + 1],
                                     acc[:, P * s:P * (s + 1)], ones_f32,
                                     start=True, stop=True,
                                     skip_group_check=True)
                recip = recip_pool.tile([P, 2], F32, tag="recip", name="recip")
                nc.vector.reciprocal(recip, den)
                # pass 2: ctx[q, d] = sum_k expS[k, q] * V[k, d], normalize.
                # s=0 skips the last two (fully masked) diagonal k-tiles.
                for s in range(2):
                    nkt = kt - 2 if s == 0 else kt
                    for n in range(2):
                        ps = p512.tile([P, 512], F32, tag="mm512", name="ps_c")
                        for t in range(nkt):
                            mm(ps,
                               expS[:, t * QB + P * s: t * QB + P * (s + 1)],
                               V_sb[:, t * D + 512 * n: t * D + 512 * (n + 1)],
                               t == 0, t == nkt - 1)
                        ot = out_pool.tile([P, 512], F32, tag="out", name="ot")
                        nc.vector.tensor_scalar_mul(ot, ps, recip[:, s:s + 1])
                        nc.sync.dma_start(
                            out=out_d[QB * jb + P * s: QB * jb + P * (s + 1),
                                      512 * n: 512 * (n + 1)],
                            in_=ot)


def build_nc():
    nc = bacc.Bacc("TRN2", target_bir_lowering=False, debug=False,
                   num_devices=8)
    xTk_d = nc.dram_tensor("xTk", [D, T // 2], BF16, kind="ExternalInput")
    xTq_d = nc.dram_tensor("xTq", [D, T // 2], BF16, kind="ExternalInput")
    wq_d = nc.dram_tensor("wq", [D, D], BF16, kind="ExternalInput")
    wk_d = nc.dram_tensor("wk", [D, D], BF16, kind="ExternalInput")
    wv_d = nc.dram_tensor("wv", [D, D], BF16, kind="ExternalInput")
    masks_d = nc.dram_tensor("masks", [4, P, QB], F32, kind="ExternalInput")
    out_d = nc.dram_tensor("out", [T // 2, D], F32, kind="ExternalOutput")
    with tile.TileContext(nc) as tc:
        _emit(nc, tc, xTk_d[:], xTq_d[:], wq_d[:], wk_d[:], wv_d[:],
              masks_d[:], out_d[:])
    nc.compile()
    return nc


def make_masks(h):
    """Additive causal mask: 0 where key (128u + p) <= query (2j + h), else
    -1e9, within a 512-position diagonal window (positions relative to the
    q-block base).  Applied to raw scores before exp."""
    u = np.arange(4)[:, None, None]
    p = np.arange(P)[None, :, None]
    j = np.arange(QB)[None, None, :]
    vis = (128 * u + p <= 2 * j + h)
    return np.where(vis, 0.0, MASK_NEG).astype(np.float32)


def make_in_maps(x, W_query, W_key, W_value):
    wq = np.ascontiguousarray(W_query).astype(BF16_NP)
    wk = np.ascontiguousarray(W_key).astype(BF16_NP)
    wv = np.ascontiguousarray(W_value).astype(BF16_NP)
    masks = [make_masks(h) for h in range(2)]
    in_maps = []
    for core in range(8):
        b, h = divmod(core, 2)
        xb = np.asarray(x[b], dtype=np.float32)
        # own 256-token blocks (2g + h for g in 0..3), packed contiguously
        own = np.concatenate(
            [xb[256 * (2 * g + h): 256 * (2 * g + h) + 256] for g in range(4)],
            axis=0)
        in_maps.append({
            "xTk": np.ascontiguousarray(own.T).astype(BF16_NP),
            "xTq": np.ascontiguousarray(xb[h::2].T).astype(BF16_NP),
            "wq": wq, "wk": wk, "wv": wv,
            "masks": masks[h],
        })
    return in_maps


_NC_CACHE = {}
LAST_EXEC_NS = None


def kernel(x, W_query, W_key, W_value):
    global LAST_EXEC_NS
    from concourse.bass_utils import run_bass_kernel_spmd

    if "nc" not in _NC_CACHE:
        _NC_CACHE["nc"] = build_nc()
    nc = _NC_CACHE["nc"]

    in_maps = make_in_maps(x, W_query, W_key, W_value)
    trace = bool(os.environ.get("BASS_TRACE"))
    res = run_bass_kernel_spmd(nc, in_maps, core_ids=list(range(8)),
                               trace=trace)
    LAST_EXEC_NS = res.exec_time_ns

    out = np.empty((B, T, D), dtype=np.float32)
    for core in range(8):
        b, h = divmod(core, 2)
        out[b, h::2, :] = res.results[core]["out"]
    return out


if __name__ == "__main__":
    import time
    t0 = time.time()
    nc = build_nc()
    print(f"build+compile took {time.time() - t0:.1f}s")
    print("built ok")
